# revision 12
# baseline (speedup 1.0000x reference)
"""Trainium2 Bass kernel for nn_Detection_Network (ROI max-pool + 4-layer head).

Strategy (8 NeuronCores, K-sharded tensor parallel):
  - Host: compute rois (int cast), a 9-map sliding-window-max pyramid of the
    feature map (window sizes {1,2,3}^2, transposed to [9*2500, C]), and the
    per-(roi, bin) pixel index table. Adaptive-max-pool bins for crops up to
    14x14 are rectangles of size <= 3x3, so each pooled value is ONE pyramid
    lookup. The lookup indices are data (not code), so the device program is
    input-independent.
  - Device core j (channels [64j, 64j+64)): dma_gather the 12544 pooled values
    (one 256B row of 64 channels per (roi, bin)), PE-transpose into featT
    tiles [(bin-pair x 64ch), 256 rois], GEMM1 against its W1 row-slice
    (K=3136) accumulating a partial h1T [4096, 256], ReduceScatter-add so core
    j owns the summed h1 rows [512j, 512j+512), GEMM2 against its W2 row-slice
    producing a partial k2T [4096, 256], then the two heads (concat, K=4096)
    producing a partial [55, 256] output.
  - Host: sum the 8 partial outputs, add the bias chain offset
    ((b1 @ W2 + b2) @ Wh + bh), transpose, emit (loc, score, rois).

Matmuls run as float32r (full PE rate, ~1e-4 rel err). All heavy compute is
on-device; host work is O(input-size) preprocessing.
"""
import sys
import os

sys.path.insert(0, "/opt/trn_rl_repo")

import numpy as np

import concourse.bass as bass
import concourse.bacc as bacc
import concourse.tile as tile
import concourse.mybir as mybir
from concourse.bass_utils import run_bass_kernel_spmd
from concourse.library_config import mlp

NUM_CORES = 8
N = 256            # rois
C = 512            # channels
H = W = 50         # feature map
POOL = 7
SCALE = 1.0 / 16.0
NBINS = POOL * POOL              # 49
CPC = C // NUM_CORES             # 64 channels per core
KLOC = NBINS * CPC               # 3136 local contraction dim
KT = 25                          # k-tiles (24 x 128 + 1 x 64)
HID = 4096
HIDC = HID // NUM_CORES          # 512 hidden rows per core
MOUT = 55                        # 44 loc + 11 score
NMAPS = 9                        # (w, h) in {1,2,3}^2
PIX = H * W                      # 2500
NIDX = N * NBINS                 # 12544 gathers per core

_cached = {}
MODE = os.environ.get("KERNEL_MODE", "full")


def _build_program():
    """Device program is shape-only; build once."""
    if "nc" in _cached:
        return _cached["nc"]

    f32 = mybir.dt.float32
    f32r = mybir.dt.float32r
    i16 = mybir.dt.int16

    nc = bacc.Bacc("TRN2", target_bir_lowering=False, debug=False,
                   num_devices=NUM_CORES)

    pyr = nc.dram_tensor("pyr", [NMAPS * PIX, CPC], f32, kind="ExternalInput")
    idxs = nc.dram_tensor("idxs", [128, NIDX // 16], i16, kind="ExternalInput")
    w1s = nc.dram_tensor("w1s", [KLOC, HID], f32r, kind="ExternalInput")
    w2s = nc.dram_tensor("w2s", [HIDC, HID], f32r, kind="ExternalInput")
    whs = nc.dram_tensor("whs", [HID, MOUT], f32r, kind="ExternalInput")
    ident_d = nc.dram_tensor("ident", [128, 128], f32, kind="ExternalInput")
    if MODE == "gather":
        out = nc.dram_tensor("out_part", [128, 2 * NBINS, CPC], f32, kind="ExternalOutput")
    elif MODE == "feat":
        out = nc.dram_tensor("out_part", [128, KT, N], f32r, kind="ExternalOutput")
    elif MODE == "gemm1":
        out = nc.dram_tensor("out_part", [128, 32, N], f32r, kind="ExternalOutput")
    elif MODE == "cc":
        out = nc.dram_tensor("out_part", [128, HIDC // 128, N], f32r, kind="ExternalOutput")
    else:
        out = nc.dram_tensor("out_part", [MOUT, N], f32, kind="ExternalOutput")

    with tile.TileContext(nc) as tc:
        with (
            tc.tile_pool(name="big", bufs=1) as big,
            tc.tile_pool(name="slab", bufs=3) as slab,
            tc.tile_pool(name="small", bufs=1) as small,
            tc.tile_pool(name="acc", bufs=8, space="PSUM") as accp,
            tc.tile_pool(name="dram", bufs=1, space="DRAM") as dr,
        ):
            nc.gpsimd.load_library(mlp)

            # ---- pooled gather: [128 (n%128), 98 (nh*49+ij), 64 (ch)] ----
            idx_t = small.tile([128, NIDX // 16], i16)
            nc.sync.dma_start(out=idx_t[:], in_=idxs[:])
            pooled = big.tile([128, 2 * NBINS, CPC], f32)
            nc.gpsimd.dma_gather(pooled[:], pyr[:], idx_t[:], NIDX, NIDX, CPC,
                                 single_packet=False)
            if MODE == "gather":
                nc.sync.dma_start(out=out[:], in_=pooled[:])

            # ---- transpose into featT [k'_local = ij*64+cl, n] tiles ----
            ident = small.tile([128, 128], f32)
            if MODE != "gather":
                nc.sync.dma_start(out=ident[:], in_=ident_d[:])
            featT = big.tile([128, KT, N], f32r, name="featT") if MODE != "gather" else None
            for ij in range(NBINS if MODE != "gather" else 0):
                rt, half = divmod(ij, 2)
                for nh in range(2):
                    tp = accp.tile([CPC, 128], f32, space="PSUM", tag="acc",
                                   name=f"tp_{ij}_{nh}")
                    nc.tensor.transpose(
                        out=tp[:],
                        in_=pooled[:, nh * NBINS + ij, :],
                        identity=ident[:],
                    )
                    nc.vector.tensor_copy(
                        out=featT[half * CPC:(half + 1) * CPC, rt,
                                  nh * 128:(nh + 1) * 128],
                        in_=tp[:],
                    )

            if MODE == "feat":
                nc.sync.dma_start(out=out[:], in_=featT[:])
            do_g1 = MODE in ("gemm1", "cc", "full")
            do_cc = MODE in ("cc", "full")
            do_g2 = MODE == "full"
            # ---- GEMM1: h1T_partial [4096, 256] = W1s.T @ featT ----
            h1sb = big.tile([128, 32, N], f32r, name="h1sb") if do_g1 else None
            for sweep in range(2 if do_g1 else 0):
                psums = [accp.tile([128, 2 * N], f32, space="PSUM", tag="acc",
                                   name=f"ps1_{sweep}_{b}") for b in range(8)]
                for rt in range(KT):
                    kr = 128 if rt < KT - 1 else KLOC - 128 * (KT - 1)
                    sl = slab.tile([128, 2048], f32r, tag="w1slab")
                    nc.sync.dma_start(
                        out=sl[:kr, :],
                        in_=w1s[128 * rt:128 * rt + kr,
                                2048 * sweep:2048 * (sweep + 1)],
                    )
                    for mm in range(16):
                        nc.tensor.matmul(
                            psums[mm // 2][:, (mm % 2) * N:(mm % 2 + 1) * N],
                            lhsT=sl[:kr, mm * 128:(mm + 1) * 128],
                            rhs=featT[:kr, rt, :],
                            start=(rt == 0 and mm % 2 == 0),
                            stop=(rt == KT - 1),
                            skip_group_check=(rt == 0 and mm % 2 == 1),
                        )
                for mm in range(16):
                    m = sweep * 16 + mm
                    nc.vector.tensor_copy(
                        out=h1sb[:, m, :],
                        in_=psums[mm // 2][:, (mm % 2) * N:(mm % 2 + 1) * N],
                    )

            if MODE == "gemm1":
                nc.sync.dma_start(out=out[:], in_=h1sb[:])
            # ---- ReduceScatter h1T over cores ----
            if do_cc:
                cc_in = dr.tile([HID, N], f32r)
                cc_out = dr.tile([HIDC, N], f32r)
                nc.sync.dma_start(
                    out=cc_in[:].rearrange("(m p) n -> p m n", p=128),
                    in_=h1sb[:],
                )
                nc.gpsimd.collective_compute(
                    "ReduceScatter",
                    mybir.AluOpType.add,
                    replica_groups=[list(range(NUM_CORES))],
                    ins=[cc_in.opt()],
                    outs=[cc_out.opt()],
                )
                h1s = big.tile([128, HIDC // 128, N], f32r)
                nc.sync.dma_start(
                    out=h1s[:],
                    in_=cc_out[:].rearrange("(t p) n -> p t n", p=128),
                )

            if MODE == "cc":
                nc.sync.dma_start(out=out[:], in_=h1s[:])
            # ---- GEMM2: k2T_partial [4096, 256] = W2s.T @ h1s ----
            k2sb = big.tile([128, 32, N], f32r, name="k2sb") if do_g2 else None
            for sweep in range(2 if do_g2 else 0):
                psums = [accp.tile([128, 2 * N], f32, space="PSUM", tag="acc",
                                   name=f"ps2_{sweep}_{b}") for b in range(8)]
                for kt in range(HIDC // 128):
                    sl = slab.tile([128, 2048], f32r, tag="w2slab")
                    nc.sync.dma_start(
                        out=sl[:],
                        in_=w2s[128 * kt:128 * (kt + 1),
                                2048 * sweep:2048 * (sweep + 1)],
                    )
                    for mm in range(16):
                        nc.tensor.matmul(
                            psums[mm // 2][:, (mm % 2) * N:(mm % 2 + 1) * N],
                            lhsT=sl[:, mm * 128:(mm + 1) * 128],
                            rhs=h1s[:, kt, :],
                            start=(kt == 0 and mm % 2 == 0),
                            stop=(kt == HIDC // 128 - 1),
                            skip_group_check=(kt == 0 and mm % 2 == 1),
                        )
                for mm in range(16):
                    m = sweep * 16 + mm
                    nc.vector.tensor_copy(
                        out=k2sb[:, m, :],
                        in_=psums[mm // 2][:, (mm % 2) * N:(mm % 2 + 1) * N],
                    )

            # ---- heads: out_partial [55, 256] = Whs.T @ k2T_partial ----
            if do_g2:
                whs_t = big.tile([128, HID // 128, MOUT], f32r)
                nc.sync.dma_start(
                    out=whs_t[:],
                    in_=whs[:].rearrange("(t p) m -> p t m", p=128),
                )
                pso = accp.tile([MOUT, N], f32, space="PSUM", tag="acc")
                for kt in range(HID // 128):
                    nc.tensor.matmul(
                        pso[:],
                        lhsT=whs_t[:, kt, :],
                        rhs=k2sb[:, kt, :],
                        start=(kt == 0),
                        stop=(kt == HID // 128 - 1),
                    )
                out_s = small.tile([MOUT, N], f32)
                nc.vector.tensor_copy(out=out_s[:], in_=pso[:])
                nc.sync.dma_start(out=out[:], in_=out_s[:])

    nc.compile()
    _cached["nc"] = nc
    return nc


def _host_prep(sample_roi, out_map):
    """rois, pyramid maps (transposed), and the gather index table."""
    sr = np.asarray(sample_roi, np.float32)
    xy = sr[:, [1, 0, 3, 2]] * SCALE                    # (x1, y1, x2, y2)
    rois = np.concatenate(
        [np.zeros((sr.shape[0], 1), np.float32), xy], axis=1
    ).astype(np.int32)                                   # trunc toward zero

    x1, y1, x2, y2 = rois[:, 1], rois[:, 2], rois[:, 3], rois[:, 4]
    w = x2 - x1 + 1
    h = y2 - y1 + 1
    j = np.arange(POOL)
    cs = x1[:, None] + (j[None, :] * w[:, None]) // POOL            # [N, 7]
    ce = x1[:, None] + (-((-(j[None, :] + 1) * w[:, None]) // POOL)) - 1
    rs = y1[:, None] + (j[None, :] * h[:, None]) // POOL
    re = y1[:, None] + (-((-(j[None, :] + 1) * h[:, None]) // POOL)) - 1
    wj = ce - cs + 1                                                # [N, 7]
    hi = re - rs + 1
    assert wj.min() >= 1 and wj.max() <= 3 and hi.min() >= 1 and hi.max() <= 3, \
        "adaptive-pool bin exceeds 3x3; pyramid fast path requires crops <= 14px"

    om = np.asarray(out_map, np.float32)[0]              # [C, 50, 50]
    # column-window maxes P_w, then row-window maxes Q_{w,h}
    P = [om]
    for wd in (2, 3):
        p = P[-1].copy()
        p[:, :, : W - (wd - 1)] = np.maximum(
            p[:, :, : W - (wd - 1)], om[:, :, wd - 1:]
        )
        P.append(p)
    pyr = np.empty((NMAPS, C, H, W), np.float32)
    for wd in (1, 2, 3):
        q = P[wd - 1]
        pyr[(1 - 1) * 3 + (wd - 1)] = q
        for ht in (2, 3):
            q = q.copy()
            q[:, : H - (ht - 1), :] = np.maximum(
                q[:, : H - (ht - 1), :], P[wd - 1][:, ht - 1:, :]
            )
            pyr[(ht - 1) * 3 + (wd - 1)] = q
    # -> [NMAPS*PIX, C] rows of channels
    pyr_t = np.ascontiguousarray(
        pyr.reshape(NMAPS, C, PIX).transpose(0, 2, 1).reshape(NMAPS * PIX, C)
    )

    # gather index for (n, i, j): g = ((n//128)*49 + i*7+j)*128 + n%128
    map_idx = (hi[:, :, None] - 1) * 3 + (wj[:, None, :] - 1)       # [N, 7i, 7j]
    pix = map_idx * PIX + rs[:, :, None] * W + cs[:, None, :]       # [N, 7i, 7j]
    pix = pix.reshape(N, NBINS)
    g_idx = np.empty(NIDX, np.int16)
    nn = np.arange(N)
    slot = (nn[:, None] // 128) * NBINS + np.arange(NBINS)[None, :]
    g = slot * 128 + (nn[:, None] % 128)
    g_idx[g.reshape(-1)] = pix.reshape(-1).astype(np.int16)
    gg = np.arange(NIDX)
    wrapped = np.empty((16, NIDX // 16), np.int16)
    wrapped[gg % 16, gg // 16] = g_idx
    idxs_np = np.tile(wrapped, (8, 1))
    return rois, pyr_t, idxs_np


def kernel(sample_roi, out_map, W1, b1, W2, b2, W_loc, b_loc, W_sc, b_sc):
    rois, pyr_t, idxs_np = _host_prep(sample_roi, out_map)

    W1 = np.asarray(W1, np.float32)
    W2 = np.asarray(W2, np.float32)
    Wh = np.concatenate(
        [np.asarray(W_loc, np.float32), np.asarray(W_sc, np.float32)], axis=1
    )

    in_maps = []
    for j in range(NUM_CORES):
        w1s = np.ascontiguousarray(
            W1.reshape(C, NBINS, HID)[CPC * j:CPC * (j + 1)]
            .transpose(1, 0, 2).reshape(KLOC, HID)
        )
        in_maps.append({
            "pyr": np.ascontiguousarray(pyr_t[:, CPC * j:CPC * (j + 1)]),
            "idxs": idxs_np,
            "w1s": w1s,
            "w2s": np.ascontiguousarray(W2[HIDC * j:HIDC * (j + 1)]),
            "whs": Wh,
            "ident": np.eye(128, dtype=np.float32),
        })

    nc = _build_program()
    res = run_bass_kernel_spmd(nc, in_maps, list(range(NUM_CORES)))

    if MODE != "full":
        return [res.results[j]["out_part"] for j in range(NUM_CORES)]

    parts = np.zeros((MOUT, N), np.float64)
    for j in range(NUM_CORES):
        parts += res.results[j]["out_part"].astype(np.float64)

    # bias chain folded out of the device program (all linear):
    b1 = np.asarray(b1, np.float64)
    b2 = np.asarray(b2, np.float64)
    bh = np.concatenate(
        [np.asarray(b_loc, np.float64), np.asarray(b_sc, np.float64)]
    )
    off = (b1 @ np.asarray(W2, np.float64) + b2) @ np.concatenate(
        [np.asarray(W_loc, np.float64), np.asarray(W_sc, np.float64)], axis=1
    ) + bh                                               # [55]

    full = parts.T + off[None, :]                        # [256, 55]
    roi_cls_loc = full[:, :44].astype(np.float32)
    roi_cls_score = full[:, 44:].astype(np.float32)
    return roi_cls_loc, roi_cls_score, rois
